# revision 15
# baseline (speedup 1.0000x reference)
import sys

if "/opt/trn_rl_repo" not in sys.path:
    sys.path.insert(0, "/opt/trn_rl_repo")

import numpy as np

import concourse.bass as bass
import concourse.bacc as bacc
import concourse.tile as tile
from concourse import mybir
from concourse.bass_utils import run_bass_kernel_spmd

M = 8               # cores
B, T, EH, H = 512, 512, 128, 64
BS = B // M         # 64 batches per core
G = 8               # batch groups per core
GB = BS // G        # 8 batches per group
F32 = mybir.dt.float32
BF16 = mybir.dt.bfloat16

_NC = None
_LAST_RESULTS = None
TRACE = False


def _build():
    nc = bacc.Bacc(num_swdge_queues=4)

    enc = nc.declare_dram_parameter("enc", [BS, T, EH], F32, isOutput=False)
    enc_diT = nc.declare_dram_parameter("enc_diT", [EH, BS], F32, isOutput=False)
    w1rep4 = nc.declare_dram_parameter("w1rep4", [128, 4 * EH], F32, isOutput=False)
    wcT = nc.declare_dram_parameter("wcT", [2, 128, H], F32, isOutput=False)
    bcomb = nc.declare_dram_parameter("bcomb", [H, 1], F32, isOutput=False)
    wihT = nc.declare_dram_parameter("wihT", [2, H, 4 * H], F32, isOutput=False)
    whhT = nc.declare_dram_parameter("whhT", [2, H, 4 * H], F32, isOutput=False)
    bihr = nc.declare_dram_parameter("bihr", [4, 128], F32, isOutput=False)
    bhhr = nc.declare_dram_parameter("bhhr", [4, 128], F32, isOutput=False)
    h0T = nc.declare_dram_parameter("h0T", [2, H, BS], F32, isOutput=False)
    c0T = nc.declare_dram_parameter("c0T", [2, H, BS], F32, isOutput=False)
    iden = nc.declare_dram_parameter("iden", [128, 128], F32, isOutput=False)
    ones = nc.declare_dram_parameter("ones", [128, 128], F32, isOutput=False)

    out0 = nc.declare_dram_parameter("out0", [BS, 1, 2 * H], F32, isOutput=True)
    out1 = nc.declare_dram_parameter("out1", [2, BS, H], F32, isOutput=True)
    out2 = nc.declare_dram_parameter("out2", [2, BS, H], F32, isOutput=True)

    Exp = mybir.ActivationFunctionType.Exp
    Sig = mybir.ActivationFunctionType.Sigmoid
    Tanh = mybir.ActivationFunctionType.Tanh
    Relu = mybir.ActivationFunctionType.Relu
    Copy = mybir.ActivationFunctionType.Copy
    PSUM = bass.MemorySpace.PSUM

    with tile.TileContext(nc) as tc:
        from contextlib import ExitStack
        with ExitStack() as ctx:
            cp = ctx.enter_context(tc.sbuf_pool(name="const", bufs=1))
            ep = ctx.enter_context(tc.sbuf_pool(name="encp", bufs=16))
            scr = ctx.enter_context(tc.sbuf_pool(name="scr", bufs=3))
            gp = ctx.enter_context(tc.sbuf_pool(name="grp", bufs=3))
            sm = ctx.enter_context(tc.sbuf_pool(name="sm", bufs=4))
            pp = ctx.enter_context(tc.psum_pool(name="ps", bufs=5))
            pp1 = ctx.enter_context(tc.psum_pool(name="ps1", bufs=3))

            # ---- constants (loaded once) ----
            w1f_sb = cp.tile([128, 4 * EH], F32, name="w1f_sb")
            nc.gpsimd.dma_start(out=w1f_sb, in_=w1rep4[:, :])
            w1bf4 = cp.tile([128, 16 * EH], BF16, name="w1bf4")
            for q in range(4):
                nc.scalar.activation(w1bf4[:, q * 4 * EH:(q + 1) * 4 * EH], w1f_sb, Copy)
            ones_sb = cp.tile([128, 128], F32, name="ones_sb")
            nc.gpsimd.dma_start(out=ones_sb, in_=ones[:, :])
            iden_sb = cp.tile([128, 128], F32, name="iden_sb")
            nc.gpsimd.dma_start(out=iden_sb, in_=iden[:, :])
            encdiT_sb = cp.tile([EH, BS], F32, name="encdiT_sb")
            nc.gpsimd.dma_start(out=encdiT_sb, in_=enc_diT[:, :])
            wcT_sb = cp.tile([128, 2 * H], F32, name="wcT_sb")
            nc.gpsimd.dma_start(out=wcT_sb[:, 0:H], in_=wcT[0, :, :])
            nc.gpsimd.dma_start(out=wcT_sb[:, H:2 * H], in_=wcT[1, :, :])
            bcomb_sb = cp.tile([H, 1], F32, name="bcomb_sb")
            nc.gpsimd.dma_start(out=bcomb_sb, in_=bcomb[:, :])
            wihT_sb = cp.tile([H, 2 * 4 * H], F32, name="wihT_sb")
            whhT_sb = cp.tile([H, 2 * 4 * H], F32, name="whhT_sb")
            for d in range(2):
                nc.gpsimd.dma_start(out=wihT_sb[:, d * 256:(d + 1) * 256], in_=wihT[d, :, :])
                nc.gpsimd.dma_start(out=whhT_sb[:, d * 256:(d + 1) * 256], in_=whhT[d, :, :])
            bihr_sb = cp.tile([1, 4 * 128], F32, name="bihr_sb")
            bhhr_sb = cp.tile([1, 4 * 128], F32, name="bhhr_sb")
            for r in range(4):
                nc.gpsimd.dma_start(out=bihr_sb[0:1, r * 128:(r + 1) * 128], in_=bihr[r:r + 1, :])
                nc.gpsimd.dma_start(out=bhhr_sb[0:1, r * 128:(r + 1) * 128], in_=bhhr[r:r + 1, :])
            h0T_sb = cp.tile([H, 2 * BS], F32, name="h0T_sb")
            c0T_sb = cp.tile([H, 2 * BS], F32, name="c0T_sb")
            for d in range(2):
                nc.gpsimd.dma_start(out=h0T_sb[:, d * BS:(d + 1) * BS], in_=h0T[d, :, :])
                nc.gpsimd.dma_start(out=c0T_sb[:, d * BS:(d + 1) * BS], in_=c0T[d, :, :])

            # xT[j, b] accumulated across groups, consumed by the LSTM tail
            xT_sb = cp.tile([H, BS], F32, name="xT_sb")

            # ---- load encoder outputs staged f32 (4 batches per DMA),
            # convert to resident bf16 tiles [128, 4*512] ----
            NB = 4  # batches per block
            enc_bf = []   # per-block bf16 tiles; batch b -> (tile b//NB, col (b%NB)*512)
            for blk in range(BS // NB):
                st = scr.tile([128, NB * 4 * EH], F32, name=f"encf_{blk}", tag="encf", bufs=8)
                dma_eng = (nc.sync, nc.scalar)[blk % 2]
                dma_eng.dma_start(
                    out=st.rearrange("p (b tc e) -> p b tc e", b=NB, tc=4),
                    in_=enc[blk * NB:(blk + 1) * NB].rearrange("b (tc p) e -> p b tc e", p=128),
                )
                bft = ep.tile([128, NB * 4 * EH], BF16, name=f"encbf_{blk}", tag="encbf")
                nc.scalar.activation(bft, st, Copy)
                enc_bf.append(bft)

            def enc_slice(b, tcc):
                # [128, 128] bf16 slice for (batch b, t-chunk tcc)
                off = (b % NB) * 4 * EH + tcc * EH
                return enc_bf[b // NB][:, off:off + EH]

            def enc_bslice(b):
                # [128, 512] bf16 slice for batch b (all 4 t-chunks)
                off = (b % NB) * 4 * EH
                return enc_bf[b // NB][:, off:off + 4 * EH]

            # ---- per-group attention pipeline ----
            for g in range(G):
                scores_g = gp.tile([128, GB * 4], F32, name=f"scores_{g}", tag="scores")
                for blk_i in range(2):          # 2 blocks of 4 batches per group
                    blk = g * 2 + blk_i
                    prod = scr.tile([128, NB * 4 * EH], BF16, name=f"prod_{blk}", tag="prod")
                    nc.vector.tensor_mul(prod, enc_bf[blk], w1bf4)
                    cur = prod
                    width = EH                   # eh elements per (b, tc)
                    while width > 8:
                        half = width // 2
                        nxt = scr.tile([128, 16 * half], BF16, name=f"tr_{blk}_{half}", tag=f"tr{half}")
                        v = cur.rearrange("p (bt two e) -> p bt two e", bt=16, two=2)
                        nc.vector.tensor_add(nxt.rearrange("p (bt e) -> p bt e", bt=16),
                                             v[:, :, 0, :], v[:, :, 1, :])
                        cur = nxt
                        width = half
                    nc.vector.tensor_reduce(
                        out=scores_g[:, blk_i * 16:(blk_i + 1) * 16],
                        in_=cur.rearrange("p (bt e) -> p bt e", bt=16),
                        axis=mybir.AxisListType.X,
                        op=mybir.AluOpType.add,
                    )
                exp_g = gp.tile([128, GB * 4], F32, name=f"exp_{g}", tag="exp")
                nc.scalar.activation(exp_g, scores_g, Exp)

                sums_ps = pp.tile([1, GB * 4], F32, name=f"sums_ps_{g}", tag="gps")
                nc.tensor.matmul(sums_ps, ones_sb[:, 0:1], exp_g, start=True, stop=True)
                sums_sb = sm.tile([1, GB * 4], F32, name=f"sums_sb_{g}", tag="sums_sb")
                nc.scalar.activation(sums_sb, sums_ps, Copy)
                totals = sm.tile([1, GB], F32, name=f"totals_{g}", tag="totals")
                nc.vector.tensor_reduce(
                    out=totals,
                    in_=sums_sb.rearrange("p (b t) -> p b t", t=4),
                    axis=mybir.AxisListType.X,
                    op=mybir.AluOpType.add,
                )
                recip = sm.tile([1, GB], F32, name=f"recip_{g}", tag="recip")
                nc.vector.reciprocal(recip, totals)
                rep_ps = pp.tile([128, GB], F32, name=f"rep_ps_{g}", tag="gps")
                nc.tensor.matmul(rep_ps, ones_sb[0:1, :], recip, start=True, stop=True)
                rep_sb = sm.tile([128, GB], F32, name=f"rep_sb_{g}", tag="rep_sb")
                nc.scalar.activation(rep_sb, rep_ps, Copy)

                p_g = gp.tile([128, GB * 4], BF16, name=f"p_{g}", tag="p")
                exp3 = exp_g.rearrange("p (b t) -> p b t", t=4)
                p3 = p_g.rearrange("p (b t) -> p b t", t=4)
                for tcc in range(4):
                    nc.vector.tensor_mul(p3[:, :, tcc], exp3[:, :, tcc], rep_sb)

                attn_ps = pp.tile([EH, GB], F32, name=f"attn_ps_{g}", tag="gps")
                for bi in range(GB):
                    b = g * GB + bi
                    for tcc in range(4):
                        nc.tensor.matmul(
                            attn_ps[:, bi:bi + 1],
                            enc_slice(b, tcc),
                            p_g[:, bi * 4 + tcc:bi * 4 + tcc + 1],
                            start=(tcc == 0),
                            stop=(tcc == 3),
                        )
                attn_sb = sm.tile([EH, GB], F32, name=f"attn_sb_{g}", tag="attn_sb")
                nc.scalar.activation(attn_sb, attn_ps, Copy)

                comb_ps = pp.tile([H, GB], F32, name=f"comb_ps_{g}", tag="gps")
                nc.tensor.matmul(comb_ps, wcT_sb[:, 0:H],
                                 encdiT_sb[:, g * GB:(g + 1) * GB], start=True, stop=False)
                nc.tensor.matmul(comb_ps, wcT_sb[:, H:2 * H],
                                 attn_sb, start=False, stop=True)
                nc.scalar.activation(xT_sb[:, g * GB:(g + 1) * GB], comb_ps, Relu,
                                     bias=bcomb_sb)

            # ---- single-step bidirectional LSTM tail ----
            for d in range(2):
                gps = []
                for ch in range(2):
                    gpp = pp1.tile([128, BS], F32, name=f"gates_{d}_{ch}", tag="lps")
                    nc.tensor.matmul(
                        gpp, wihT_sb[:, d * 256 + ch * 128:d * 256 + (ch + 1) * 128],
                        xT_sb, start=True, stop=False)
                    nc.tensor.matmul(
                        gpp, whhT_sb[:, d * 256 + ch * 128:d * 256 + (ch + 1) * 128],
                        h0T_sb[:, d * BS:(d + 1) * BS], start=False, stop=False)
                    r = d * 2 + ch
                    nc.tensor.matmul(gpp, bihr_sb[0:1, r * 128:(r + 1) * 128], ones_sb[0:1, 0:BS],
                                     start=False, stop=False)
                    nc.tensor.matmul(gpp, bhhr_sb[0:1, r * 128:(r + 1) * 128], ones_sb[0:1, 0:BS],
                                     start=False, stop=True)
                    gps.append(gpp)
                sigi = sm.tile([H, BS], F32, name=f"sigi_{d}", tag="sigi")
                nc.scalar.activation(sigi, gps[0][0:H, :], Sig)
                sigf = sm.tile([H, BS], F32, name=f"sigf_{d}", tag="sigf")
                nc.scalar.activation(sigf, gps[0][H:2 * H, :], Sig)
                tanhg = sm.tile([H, BS], F32, name=f"tanhg_{d}", tag="tanhg")
                nc.scalar.activation(tanhg, gps[1][0:H, :], Tanh)
                sigo = sm.tile([H, BS], F32, name=f"sigo_{d}", tag="sigo")
                nc.scalar.activation(sigo, gps[1][H:2 * H, :], Sig)

                t1 = sm.tile([H, BS], F32, name=f"t1_{d}", tag="t1")
                nc.vector.tensor_mul(t1, sigf, c0T_sb[:, d * BS:(d + 1) * BS])
                t2 = sm.tile([H, BS], F32, name=f"t2_{d}", tag="t2")
                nc.vector.tensor_mul(t2, sigi, tanhg)
                cT = sm.tile([H, BS], F32, name=f"cT_{d}", tag="cT")
                nc.vector.tensor_add(cT, t1, t2)
                tanhc = sm.tile([H, BS], F32, name=f"tanhc_{d}", tag="tanhc")
                nc.scalar.activation(tanhc, cT, Tanh)
                hT = sm.tile([H, BS], F32, name=f"hT_{d}", tag="hT")
                nc.vector.tensor_mul(hT, sigo, tanhc)

                # transpose back to [b, h] and write outputs
                for src, dst in ((hT, None), (cT, out2)):
                    tp = pp1.tile([BS, H], F32, name=f"tp_{d}_{0 if dst is None else 1}", tag="lps")
                    nc.tensor.transpose(tp, src, iden_sb[0:H, 0:H])
                    nat = sm.tile([BS, H], F32, name=f"nat_{d}_{0 if dst is None else 1}", tag="nat")
                    nc.scalar.activation(nat, tp, Copy)
                    if dst is None:
                        nc.sync.dma_start(out=out1[d, :, :], in_=nat)
                        nc.sync.dma_start(out=out0[:, 0, d * H:(d + 1) * H], in_=nat)
                    else:
                        nc.sync.dma_start(out=dst[d, :, :], in_=nat)

    nc.finalize()
    return nc


def kernel(h0, c0, encoder_outputs, W_attn, b_attn, W_comb, b_comb,
           w_ih_f, w_hh_f, b_ih_f, b_hh_f, w_ih_r, w_hh_r, b_ih_r, b_hh_r, di):
    global _NC, _LAST_RESULTS
    f = np.float32
    h0 = np.asarray(h0, f)
    c0 = np.asarray(c0, f)
    enc = np.ascontiguousarray(np.asarray(encoder_outputs, f))
    di_i = int(np.asarray(di))

    w1rep4 = np.ascontiguousarray(np.broadcast_to(np.tile(np.asarray(W_attn, f)[0, :EH], 4), (128, 4 * EH)))
    wcT = np.ascontiguousarray(np.asarray(W_comb, f).T.reshape(2, 128, H))
    bcomb = np.ascontiguousarray(np.asarray(b_comb, f).reshape(H, 1))
    wihT = np.ascontiguousarray(np.stack([np.asarray(w_ih_f, f).T, np.asarray(w_ih_r, f).T]))
    whhT = np.ascontiguousarray(np.stack([np.asarray(w_hh_f, f).T, np.asarray(w_hh_r, f).T]))

    def pack_bias(bf, br):
        return np.ascontiguousarray(
            np.stack([np.asarray(bf, f), np.asarray(br, f)]).reshape(4, 128))

    bihr = pack_bias(b_ih_f, b_ih_r)
    bhhr = pack_bias(b_hh_f, b_hh_r)
    h0T = np.ascontiguousarray(np.transpose(h0, (0, 2, 1)))   # (2, H, B)
    c0T = np.ascontiguousarray(np.transpose(c0, (0, 2, 1)))
    enc_diT = np.ascontiguousarray(enc[:, di_i, :].T)         # (EH, B)
    iden = np.eye(128, dtype=f)
    onesm = np.ones((128, 128), f)

    if _NC is None:
        _NC = _build()

    in_maps = []
    for i in range(M):
        sl = slice(i * BS, (i + 1) * BS)
        in_maps.append(dict(
            enc=np.ascontiguousarray(enc[sl]),
            enc_diT=np.ascontiguousarray(enc_diT[:, sl]),
            w1rep4=w1rep4, wcT=wcT, bcomb=bcomb,
            wihT=wihT, whhT=whhT, bihr=bihr, bhhr=bhhr,
            h0T=np.ascontiguousarray(h0T[:, :, sl]),
            c0T=np.ascontiguousarray(c0T[:, :, sl]),
            iden=iden, ones=onesm,
        ))

    _LAST_RESULTS = run_bass_kernel_spmd(
        _NC, in_maps, core_ids=list(range(M)), trace=TRACE)
    res = _LAST_RESULTS.results

    output = np.concatenate([res[i]["out0"] for i in range(M)], axis=0)
    h_new = np.concatenate([res[i]["out1"] for i in range(M)], axis=1)
    c_new = np.concatenate([res[i]["out2"] for i in range(M)], axis=1)
    return output, h_new, c_new


# revision 16
# speedup vs baseline: 1.0760x; 1.0760x over previous
import sys

if "/opt/trn_rl_repo" not in sys.path:
    sys.path.insert(0, "/opt/trn_rl_repo")

import numpy as np

import concourse.bass as bass
import concourse.bacc as bacc
import concourse.tile as tile
from concourse import mybir
from concourse.bass_utils import run_bass_kernel_spmd

M = 8               # cores
B, T, EH, H = 512, 512, 128, 64
BS = B // M         # 64 batches per core
G = 8               # batch groups per core
GB = BS // G        # 8 batches per group
F32 = mybir.dt.float32
BF16 = mybir.dt.bfloat16

_NC = None
_LAST_RESULTS = None
TRACE = False


def _build():
    nc = bacc.Bacc(num_swdge_queues=4)

    enc = nc.declare_dram_parameter("enc", [BS, T, EH], F32, isOutput=False)
    enc_diT = nc.declare_dram_parameter("enc_diT", [EH, BS], F32, isOutput=False)
    w1rep4 = nc.declare_dram_parameter("w1rep4", [128, 4 * EH], F32, isOutput=False)
    wcT = nc.declare_dram_parameter("wcT", [2, 128, H], F32, isOutput=False)
    bcomb = nc.declare_dram_parameter("bcomb", [H, 1], F32, isOutput=False)
    wihT = nc.declare_dram_parameter("wihT", [2, H, 4 * H], F32, isOutput=False)
    whhT = nc.declare_dram_parameter("whhT", [2, H, 4 * H], F32, isOutput=False)
    bihr = nc.declare_dram_parameter("bihr", [4, 128], F32, isOutput=False)
    bhhr = nc.declare_dram_parameter("bhhr", [4, 128], F32, isOutput=False)
    h0T = nc.declare_dram_parameter("h0T", [2, H, BS], F32, isOutput=False)
    c0T = nc.declare_dram_parameter("c0T", [2, H, BS], F32, isOutput=False)
    iden = nc.declare_dram_parameter("iden", [128, 128], F32, isOutput=False)
    ones = nc.declare_dram_parameter("ones", [128, 128], F32, isOutput=False)

    out0 = nc.declare_dram_parameter("out0", [BS, 1, 2 * H], F32, isOutput=True)
    out1 = nc.declare_dram_parameter("out1", [2, BS, H], F32, isOutput=True)
    out2 = nc.declare_dram_parameter("out2", [2, BS, H], F32, isOutput=True)

    Exp = mybir.ActivationFunctionType.Exp
    Sig = mybir.ActivationFunctionType.Sigmoid
    Tanh = mybir.ActivationFunctionType.Tanh
    Relu = mybir.ActivationFunctionType.Relu
    Copy = mybir.ActivationFunctionType.Copy
    PSUM = bass.MemorySpace.PSUM

    with tile.TileContext(nc) as tc:
        from contextlib import ExitStack
        with ExitStack() as ctx:
            cp = ctx.enter_context(tc.sbuf_pool(name="const", bufs=1))
            ep = ctx.enter_context(tc.sbuf_pool(name="encp", bufs=16))
            scr = ctx.enter_context(tc.sbuf_pool(name="scr", bufs=3))
            gp = ctx.enter_context(tc.sbuf_pool(name="grp", bufs=3))
            sm = ctx.enter_context(tc.sbuf_pool(name="sm", bufs=4))
            pp = ctx.enter_context(tc.psum_pool(name="ps", bufs=5))
            pp1 = ctx.enter_context(tc.psum_pool(name="ps1", bufs=3))

            # ---- constants (loaded once) ----
            w1f_sb = cp.tile([128, 4 * EH], F32, name="w1f_sb")
            nc.gpsimd.dma_start(out=w1f_sb, in_=w1rep4[:, :])
            w1bf4 = cp.tile([128, 16 * EH], BF16, name="w1bf4")
            for q in range(4):
                nc.scalar.activation(w1bf4[:, q * 4 * EH:(q + 1) * 4 * EH], w1f_sb, Copy)
            ones_sb = cp.tile([128, 128], F32, name="ones_sb")
            nc.gpsimd.dma_start(out=ones_sb, in_=ones[:, :])
            iden_sb = cp.tile([128, 128], F32, name="iden_sb")
            nc.gpsimd.dma_start(out=iden_sb, in_=iden[:, :])
            encdiT_sb = cp.tile([EH, BS], F32, name="encdiT_sb")
            nc.gpsimd.dma_start(out=encdiT_sb, in_=enc_diT[:, :])
            wcT_sb = cp.tile([128, 2 * H], F32, name="wcT_sb")
            nc.gpsimd.dma_start(out=wcT_sb[:, 0:H], in_=wcT[0, :, :])
            nc.gpsimd.dma_start(out=wcT_sb[:, H:2 * H], in_=wcT[1, :, :])
            bcomb_sb = cp.tile([H, 1], F32, name="bcomb_sb")
            nc.gpsimd.dma_start(out=bcomb_sb, in_=bcomb[:, :])
            wihT_sb = cp.tile([H, 2 * 4 * H], F32, name="wihT_sb")
            whhT_sb = cp.tile([H, 2 * 4 * H], F32, name="whhT_sb")
            for d in range(2):
                nc.gpsimd.dma_start(out=wihT_sb[:, d * 256:(d + 1) * 256], in_=wihT[d, :, :])
                nc.gpsimd.dma_start(out=whhT_sb[:, d * 256:(d + 1) * 256], in_=whhT[d, :, :])
            bihr_sb = cp.tile([1, 4 * 128], F32, name="bihr_sb")
            bhhr_sb = cp.tile([1, 4 * 128], F32, name="bhhr_sb")
            for r in range(4):
                nc.gpsimd.dma_start(out=bihr_sb[0:1, r * 128:(r + 1) * 128], in_=bihr[r:r + 1, :])
                nc.gpsimd.dma_start(out=bhhr_sb[0:1, r * 128:(r + 1) * 128], in_=bhhr[r:r + 1, :])
            h0T_sb = cp.tile([H, 2 * BS], F32, name="h0T_sb")
            c0T_sb = cp.tile([H, 2 * BS], F32, name="c0T_sb")
            for d in range(2):
                nc.gpsimd.dma_start(out=h0T_sb[:, d * BS:(d + 1) * BS], in_=h0T[d, :, :])
                nc.gpsimd.dma_start(out=c0T_sb[:, d * BS:(d + 1) * BS], in_=c0T[d, :, :])

            # xT[j, b] accumulated across groups, consumed by the LSTM tail
            xT_sb = cp.tile([H, BS], F32, name="xT_sb")

            # ---- load encoder outputs staged f32 (4 batches per DMA),
            # convert to resident bf16 tiles [128, 4*512] ----
            NB = 4  # batches per block
            enc_bf = []   # per-block bf16 tiles; batch b -> (tile b//NB, col (b%NB)*512)
            for blk in range(BS // NB):
                st = scr.tile([128, NB * 4 * EH], F32, name=f"encf_{blk}", tag="encf", bufs=8)
                dma_eng = (nc.gpsimd, nc.sync)[blk % 2]
                dma_eng.dma_start(
                    out=st.rearrange("p (b tc e) -> p b tc e", b=NB, tc=4),
                    in_=enc[blk * NB:(blk + 1) * NB].rearrange("b (tc p) e -> p b tc e", p=128),
                )
                bft = ep.tile([128, NB * 4 * EH], BF16, name=f"encbf_{blk}", tag="encbf")
                enc_bf.append((st, bft))

            def enc_slice(b, tcc):
                # [128, 128] bf16 slice for (batch b, t-chunk tcc)
                off = (b % NB) * 4 * EH + tcc * EH
                return enc_bf[b // NB][1][:, off:off + EH]

            # ---- per-group attention pipeline ----
            for g in range(G):
                scores_g = gp.tile([128, GB * 4], F32, name=f"scores_{g}", tag="scores")
                for blk_i in range(2):          # 2 blocks of 4 batches per group
                    blk = g * 2 + blk_i
                    st, bft = enc_bf[blk]
                    nc.scalar.activation(bft, st, Copy)
                    prod = scr.tile([128, NB * 4 * EH], BF16, name=f"prod_{blk}", tag="prod")
                    nc.vector.tensor_mul(prod, bft, w1bf4)
                    cur = prod
                    width = EH                   # eh elements per (b, tc)
                    while width > 8:
                        half = width // 2
                        nxt = scr.tile([128, 16 * half], BF16, name=f"tr_{blk}_{half}", tag=f"tr{half}")
                        v = cur.rearrange("p (bt two e) -> p bt two e", bt=16, two=2)
                        nc.vector.tensor_add(nxt.rearrange("p (bt e) -> p bt e", bt=16),
                                             v[:, :, 0, :], v[:, :, 1, :])
                        cur = nxt
                        width = half
                    nc.vector.tensor_reduce(
                        out=scores_g[:, blk_i * 16:(blk_i + 1) * 16],
                        in_=cur.rearrange("p (bt e) -> p bt e", bt=16),
                        axis=mybir.AxisListType.X,
                        op=mybir.AluOpType.add,
                    )
                exp_g = gp.tile([128, GB * 4], F32, name=f"exp_{g}", tag="exp")
                nc.scalar.activation(exp_g, scores_g, Exp)

                sums_ps = pp.tile([1, GB * 4], F32, name=f"sums_ps_{g}", tag="gps")
                nc.tensor.matmul(sums_ps, ones_sb[:, 0:1], exp_g, start=True, stop=True)
                sums_sb = sm.tile([1, GB * 4], F32, name=f"sums_sb_{g}", tag="sums_sb")
                nc.scalar.activation(sums_sb, sums_ps, Copy)
                totals = sm.tile([1, GB], F32, name=f"totals_{g}", tag="totals")
                nc.vector.tensor_reduce(
                    out=totals,
                    in_=sums_sb.rearrange("p (b t) -> p b t", t=4),
                    axis=mybir.AxisListType.X,
                    op=mybir.AluOpType.add,
                )
                recip = sm.tile([1, GB], F32, name=f"recip_{g}", tag="recip")
                nc.vector.reciprocal(recip, totals)
                rep_ps = pp.tile([128, GB], F32, name=f"rep_ps_{g}", tag="gps")
                nc.tensor.matmul(rep_ps, ones_sb[0:1, :], recip, start=True, stop=True)
                rep_sb = sm.tile([128, GB], F32, name=f"rep_sb_{g}", tag="rep_sb")
                nc.scalar.activation(rep_sb, rep_ps, Copy)

                p_g = gp.tile([128, GB * 4], BF16, name=f"p_{g}", tag="p")
                exp3 = exp_g.rearrange("p (b t) -> p b t", t=4)
                p3 = p_g.rearrange("p (b t) -> p b t", t=4)
                for tcc in range(4):
                    nc.vector.tensor_mul(p3[:, :, tcc], exp3[:, :, tcc], rep_sb)

                attn_ps = pp.tile([EH, GB], F32, name=f"attn_ps_{g}", tag="gps")
                for bi in range(GB):
                    b = g * GB + bi
                    for tcc in range(4):
                        nc.tensor.matmul(
                            attn_ps[:, bi:bi + 1],
                            enc_slice(b, tcc),
                            p_g[:, bi * 4 + tcc:bi * 4 + tcc + 1],
                            start=(tcc == 0),
                            stop=(tcc == 3),
                        )
                attn_sb = sm.tile([EH, GB], F32, name=f"attn_sb_{g}", tag="attn_sb")
                nc.scalar.activation(attn_sb, attn_ps, Copy)

                comb_ps = pp.tile([H, GB], F32, name=f"comb_ps_{g}", tag="gps")
                nc.tensor.matmul(comb_ps, wcT_sb[:, 0:H],
                                 encdiT_sb[:, g * GB:(g + 1) * GB], start=True, stop=False)
                nc.tensor.matmul(comb_ps, wcT_sb[:, H:2 * H],
                                 attn_sb, start=False, stop=True)
                nc.scalar.activation(xT_sb[:, g * GB:(g + 1) * GB], comb_ps, Relu,
                                     bias=bcomb_sb)

            # ---- single-step bidirectional LSTM tail ----
            for d in range(2):
                gps = []
                for ch in range(2):
                    gpp = pp1.tile([128, BS], F32, name=f"gates_{d}_{ch}", tag="lps")
                    nc.tensor.matmul(
                        gpp, wihT_sb[:, d * 256 + ch * 128:d * 256 + (ch + 1) * 128],
                        xT_sb, start=True, stop=False)
                    nc.tensor.matmul(
                        gpp, whhT_sb[:, d * 256 + ch * 128:d * 256 + (ch + 1) * 128],
                        h0T_sb[:, d * BS:(d + 1) * BS], start=False, stop=False)
                    r = d * 2 + ch
                    nc.tensor.matmul(gpp, bihr_sb[0:1, r * 128:(r + 1) * 128], ones_sb[0:1, 0:BS],
                                     start=False, stop=False)
                    nc.tensor.matmul(gpp, bhhr_sb[0:1, r * 128:(r + 1) * 128], ones_sb[0:1, 0:BS],
                                     start=False, stop=True)
                    gps.append(gpp)
                sigi = sm.tile([H, BS], F32, name=f"sigi_{d}", tag="sigi")
                nc.scalar.activation(sigi, gps[0][0:H, :], Sig)
                sigf = sm.tile([H, BS], F32, name=f"sigf_{d}", tag="sigf")
                nc.scalar.activation(sigf, gps[0][H:2 * H, :], Sig)
                tanhg = sm.tile([H, BS], F32, name=f"tanhg_{d}", tag="tanhg")
                nc.scalar.activation(tanhg, gps[1][0:H, :], Tanh)
                sigo = sm.tile([H, BS], F32, name=f"sigo_{d}", tag="sigo")
                nc.scalar.activation(sigo, gps[1][H:2 * H, :], Sig)

                t1 = sm.tile([H, BS], F32, name=f"t1_{d}", tag="t1")
                nc.vector.tensor_mul(t1, sigf, c0T_sb[:, d * BS:(d + 1) * BS])
                t2 = sm.tile([H, BS], F32, name=f"t2_{d}", tag="t2")
                nc.vector.tensor_mul(t2, sigi, tanhg)
                cT = sm.tile([H, BS], F32, name=f"cT_{d}", tag="cT")
                nc.vector.tensor_add(cT, t1, t2)
                tanhc = sm.tile([H, BS], F32, name=f"tanhc_{d}", tag="tanhc")
                nc.scalar.activation(tanhc, cT, Tanh)
                hT = sm.tile([H, BS], F32, name=f"hT_{d}", tag="hT")
                nc.vector.tensor_mul(hT, sigo, tanhc)

                # transpose back to [b, h] and write outputs
                for src, dst in ((hT, None), (cT, out2)):
                    tp = pp1.tile([BS, H], F32, name=f"tp_{d}_{0 if dst is None else 1}", tag="lps")
                    nc.tensor.transpose(tp, src, iden_sb[0:H, 0:H])
                    nat = sm.tile([BS, H], F32, name=f"nat_{d}_{0 if dst is None else 1}", tag="nat")
                    nc.scalar.activation(nat, tp, Copy)
                    if dst is None:
                        nc.sync.dma_start(out=out1[d, :, :], in_=nat)
                        nc.sync.dma_start(out=out0[:, 0, d * H:(d + 1) * H], in_=nat)
                    else:
                        nc.sync.dma_start(out=dst[d, :, :], in_=nat)

    nc.finalize()
    return nc


def kernel(h0, c0, encoder_outputs, W_attn, b_attn, W_comb, b_comb,
           w_ih_f, w_hh_f, b_ih_f, b_hh_f, w_ih_r, w_hh_r, b_ih_r, b_hh_r, di):
    global _NC, _LAST_RESULTS
    f = np.float32
    h0 = np.asarray(h0, f)
    c0 = np.asarray(c0, f)
    enc = np.ascontiguousarray(np.asarray(encoder_outputs, f))
    di_i = int(np.asarray(di))

    w1rep4 = np.ascontiguousarray(np.broadcast_to(np.tile(np.asarray(W_attn, f)[0, :EH], 4), (128, 4 * EH)))
    wcT = np.ascontiguousarray(np.asarray(W_comb, f).T.reshape(2, 128, H))
    bcomb = np.ascontiguousarray(np.asarray(b_comb, f).reshape(H, 1))
    wihT = np.ascontiguousarray(np.stack([np.asarray(w_ih_f, f).T, np.asarray(w_ih_r, f).T]))
    whhT = np.ascontiguousarray(np.stack([np.asarray(w_hh_f, f).T, np.asarray(w_hh_r, f).T]))

    def pack_bias(bf, br):
        return np.ascontiguousarray(
            np.stack([np.asarray(bf, f), np.asarray(br, f)]).reshape(4, 128))

    bihr = pack_bias(b_ih_f, b_ih_r)
    bhhr = pack_bias(b_hh_f, b_hh_r)
    h0T = np.ascontiguousarray(np.transpose(h0, (0, 2, 1)))   # (2, H, B)
    c0T = np.ascontiguousarray(np.transpose(c0, (0, 2, 1)))
    enc_diT = np.ascontiguousarray(enc[:, di_i, :].T)         # (EH, B)
    iden = np.eye(128, dtype=f)
    onesm = np.ones((128, 128), f)

    if _NC is None:
        _NC = _build()

    in_maps = []
    for i in range(M):
        sl = slice(i * BS, (i + 1) * BS)
        in_maps.append(dict(
            enc=np.ascontiguousarray(enc[sl]),
            enc_diT=np.ascontiguousarray(enc_diT[:, sl]),
            w1rep4=w1rep4, wcT=wcT, bcomb=bcomb,
            wihT=wihT, whhT=whhT, bihr=bihr, bhhr=bhhr,
            h0T=np.ascontiguousarray(h0T[:, :, sl]),
            c0T=np.ascontiguousarray(c0T[:, :, sl]),
            iden=iden, ones=onesm,
        ))

    _LAST_RESULTS = run_bass_kernel_spmd(
        _NC, in_maps, core_ids=list(range(M)), trace=TRACE)
    res = _LAST_RESULTS.results

    output = np.concatenate([res[i]["out0"] for i in range(M)], axis=0)
    h_new = np.concatenate([res[i]["out1"] for i in range(M)], axis=1)
    c_new = np.concatenate([res[i]["out2"] for i in range(M)], axis=1)
    return output, h_new, c_new


# revision 17
# speedup vs baseline: 1.1740x; 1.0910x over previous
import sys

if "/opt/trn_rl_repo" not in sys.path:
    sys.path.insert(0, "/opt/trn_rl_repo")

import numpy as np

import concourse.bass as bass
import concourse.bacc as bacc
import concourse.tile as tile
from concourse import mybir
from concourse.bass_utils import run_bass_kernel_spmd

M = 8               # cores
B, T, EH, H = 512, 512, 128, 64
BS = B // M         # 64 batches per core
G = 8               # batch groups per core
GB = BS // G        # 8 batches per group
F32 = mybir.dt.float32
BF16 = mybir.dt.bfloat16

_NC = None
_LAST_RESULTS = None
TRACE = False


def _build():
    nc = bacc.Bacc(num_swdge_queues=4)

    enc = nc.declare_dram_parameter("enc", [BS, T, EH], F32, isOutput=False)
    enc_diT = nc.declare_dram_parameter("enc_diT", [EH, BS], F32, isOutput=False)
    w1rep4 = nc.declare_dram_parameter("w1rep4", [128, 4 * EH], F32, isOutput=False)
    wcT = nc.declare_dram_parameter("wcT", [2, 128, H], F32, isOutput=False)
    bcomb = nc.declare_dram_parameter("bcomb", [H, 1], F32, isOutput=False)
    wihT = nc.declare_dram_parameter("wihT", [2, H, 4 * H], F32, isOutput=False)
    whhT = nc.declare_dram_parameter("whhT", [2, H, 4 * H], F32, isOutput=False)
    bihr = nc.declare_dram_parameter("bihr", [4, 128], F32, isOutput=False)
    bhhr = nc.declare_dram_parameter("bhhr", [4, 128], F32, isOutput=False)
    h0T = nc.declare_dram_parameter("h0T", [2, H, BS], F32, isOutput=False)
    c0T = nc.declare_dram_parameter("c0T", [2, H, BS], F32, isOutput=False)
    iden = nc.declare_dram_parameter("iden", [128, 128], F32, isOutput=False)
    ones = nc.declare_dram_parameter("ones", [128, 128], F32, isOutput=False)

    out0 = nc.declare_dram_parameter("out0", [BS, 1, 2 * H], F32, isOutput=True)
    out1 = nc.declare_dram_parameter("out1", [2, BS, H], F32, isOutput=True)
    out2 = nc.declare_dram_parameter("out2", [2, BS, H], F32, isOutput=True)

    Exp = mybir.ActivationFunctionType.Exp
    Sig = mybir.ActivationFunctionType.Sigmoid
    Tanh = mybir.ActivationFunctionType.Tanh
    Relu = mybir.ActivationFunctionType.Relu
    Copy = mybir.ActivationFunctionType.Copy
    PSUM = bass.MemorySpace.PSUM

    with tile.TileContext(nc) as tc:
        from contextlib import ExitStack
        with ExitStack() as ctx:
            cp = ctx.enter_context(tc.sbuf_pool(name="const", bufs=1))
            ep = ctx.enter_context(tc.sbuf_pool(name="encp", bufs=16))
            scr = ctx.enter_context(tc.sbuf_pool(name="scr", bufs=3))
            gp = ctx.enter_context(tc.sbuf_pool(name="grp", bufs=3))
            sm = ctx.enter_context(tc.sbuf_pool(name="sm", bufs=4))
            pp = ctx.enter_context(tc.psum_pool(name="ps", bufs=5))
            pp1 = ctx.enter_context(tc.psum_pool(name="ps1", bufs=3))

            # ---- constants (loaded once) ----
            w1f_sb = cp.tile([128, 4 * EH], F32, name="w1f_sb")
            nc.gpsimd.dma_start(out=w1f_sb, in_=w1rep4[:, :])
            w1bf4 = cp.tile([128, 16 * EH], BF16, name="w1bf4")
            for q in range(4):
                nc.scalar.activation(w1bf4[:, q * 4 * EH:(q + 1) * 4 * EH], w1f_sb, Copy)
            ones_sb = cp.tile([128, 128], F32, name="ones_sb")
            nc.gpsimd.dma_start(out=ones_sb, in_=ones[:, :])
            iden_sb = cp.tile([128, 128], F32, name="iden_sb")
            nc.gpsimd.dma_start(out=iden_sb, in_=iden[:, :])
            encdiT_sb = cp.tile([EH, BS], F32, name="encdiT_sb")
            nc.gpsimd.dma_start(out=encdiT_sb, in_=enc_diT[:, :])
            wcT_sb = cp.tile([128, 2 * H], F32, name="wcT_sb")
            nc.gpsimd.dma_start(out=wcT_sb[:, 0:H], in_=wcT[0, :, :])
            nc.gpsimd.dma_start(out=wcT_sb[:, H:2 * H], in_=wcT[1, :, :])
            bcomb_sb = cp.tile([H, 1], F32, name="bcomb_sb")
            nc.gpsimd.dma_start(out=bcomb_sb, in_=bcomb[:, :])
            wihT_sb = cp.tile([H, 2 * 4 * H], F32, name="wihT_sb")
            whhT_sb = cp.tile([H, 2 * 4 * H], F32, name="whhT_sb")
            for d in range(2):
                nc.gpsimd.dma_start(out=wihT_sb[:, d * 256:(d + 1) * 256], in_=wihT[d, :, :])
                nc.gpsimd.dma_start(out=whhT_sb[:, d * 256:(d + 1) * 256], in_=whhT[d, :, :])
            bihr_sb = cp.tile([1, 4 * 128], F32, name="bihr_sb")
            bhhr_sb = cp.tile([1, 4 * 128], F32, name="bhhr_sb")
            for r in range(4):
                nc.gpsimd.dma_start(out=bihr_sb[0:1, r * 128:(r + 1) * 128], in_=bihr[r:r + 1, :])
                nc.gpsimd.dma_start(out=bhhr_sb[0:1, r * 128:(r + 1) * 128], in_=bhhr[r:r + 1, :])
            h0T_sb = cp.tile([H, 2 * BS], F32, name="h0T_sb")
            c0T_sb = cp.tile([H, 2 * BS], F32, name="c0T_sb")
            for d in range(2):
                nc.gpsimd.dma_start(out=h0T_sb[:, d * BS:(d + 1) * BS], in_=h0T[d, :, :])
                nc.gpsimd.dma_start(out=c0T_sb[:, d * BS:(d + 1) * BS], in_=c0T[d, :, :])

            # xT[j, b] accumulated across groups, consumed by the LSTM tail
            xT_sb = cp.tile([H, BS], F32, name="xT_sb")

            # ---- load encoder outputs staged f32 (4 batches per DMA),
            # convert to resident bf16 tiles [128, 4*512] ----
            NB = 4  # batches per block
            enc_bf = []   # per-block bf16 tiles; batch b -> (tile b//NB, col (b%NB)*512)
            for blk in range(BS // NB):
                st = scr.tile([128, NB * 4 * EH], F32, name=f"encf_{blk}", tag="encf", bufs=8)
                dma_eng = (nc.gpsimd, nc.sync)[blk % 2]
                dma_eng.dma_start(
                    out=st.rearrange("p (b tc e) -> p b tc e", b=NB, tc=4),
                    in_=enc[blk * NB:(blk + 1) * NB].rearrange("b (p tc) e -> p b tc e", p=128),
                )
                bft = ep.tile([128, NB * 4 * EH], BF16, name=f"encbf_{blk}", tag="encbf")
                enc_bf.append((st, bft))

            def enc_slice(b, tcc):
                # [128, 128] bf16 slice for (batch b, t-chunk tcc)
                off = (b % NB) * 4 * EH + tcc * EH
                return enc_bf[b // NB][1][:, off:off + EH]

            # ---- per-group attention pipeline ----
            for g in range(G):
                scores_g = gp.tile([128, GB * 4], F32, name=f"scores_{g}", tag="scores")
                for blk_i in range(2):          # 2 blocks of 4 batches per group
                    blk = g * 2 + blk_i
                    st, bft = enc_bf[blk]
                    nc.scalar.activation(bft, st, Copy)
                    prod = scr.tile([128, NB * 4 * EH], BF16, name=f"prod_{blk}", tag="prod")
                    nc.vector.tensor_mul(prod, bft, w1bf4)
                    cur = prod
                    width = EH                   # eh elements per (b, tc)
                    while width > 8:
                        half = width // 2
                        nxt = scr.tile([128, 16 * half], BF16, name=f"tr_{blk}_{half}", tag=f"tr{half}")
                        v = cur.rearrange("p (bt two e) -> p bt two e", bt=16, two=2)
                        nc.vector.tensor_add(nxt.rearrange("p (bt e) -> p bt e", bt=16),
                                             v[:, :, 0, :], v[:, :, 1, :])
                        cur = nxt
                        width = half
                    nc.vector.tensor_reduce(
                        out=scores_g[:, blk_i * 16:(blk_i + 1) * 16],
                        in_=cur.rearrange("p (bt e) -> p bt e", bt=16),
                        axis=mybir.AxisListType.X,
                        op=mybir.AluOpType.add,
                    )
                exp_g = gp.tile([128, GB * 4], F32, name=f"exp_{g}", tag="exp")
                nc.scalar.activation(exp_g, scores_g, Exp)

                sums_ps = pp.tile([1, GB * 4], F32, name=f"sums_ps_{g}", tag="gps")
                nc.tensor.matmul(sums_ps, ones_sb[:, 0:1], exp_g, start=True, stop=True)
                sums_sb = sm.tile([1, GB * 4], F32, name=f"sums_sb_{g}", tag="sums_sb")
                nc.scalar.activation(sums_sb, sums_ps, Copy)
                totals = sm.tile([1, GB], F32, name=f"totals_{g}", tag="totals")
                nc.vector.tensor_reduce(
                    out=totals,
                    in_=sums_sb.rearrange("p (b t) -> p b t", t=4),
                    axis=mybir.AxisListType.X,
                    op=mybir.AluOpType.add,
                )
                recip = sm.tile([1, GB], F32, name=f"recip_{g}", tag="recip")
                nc.vector.reciprocal(recip, totals)
                rep_ps = pp.tile([128, GB], F32, name=f"rep_ps_{g}", tag="gps")
                nc.tensor.matmul(rep_ps, ones_sb[0:1, :], recip, start=True, stop=True)
                rep_sb = sm.tile([128, GB], F32, name=f"rep_sb_{g}", tag="rep_sb")
                nc.scalar.activation(rep_sb, rep_ps, Copy)

                p_g = gp.tile([128, GB * 4], BF16, name=f"p_{g}", tag="p")
                exp3 = exp_g.rearrange("p (b t) -> p b t", t=4)
                p3 = p_g.rearrange("p (b t) -> p b t", t=4)
                for tcc in range(4):
                    nc.vector.tensor_mul(p3[:, :, tcc], exp3[:, :, tcc], rep_sb)

                attn_ps = pp.tile([EH, GB], F32, name=f"attn_ps_{g}", tag="gps")
                for bi in range(GB):
                    b = g * GB + bi
                    for tcc in range(4):
                        nc.tensor.matmul(
                            attn_ps[:, bi:bi + 1],
                            enc_slice(b, tcc),
                            p_g[:, bi * 4 + tcc:bi * 4 + tcc + 1],
                            start=(tcc == 0),
                            stop=(tcc == 3),
                        )
                attn_sb = sm.tile([EH, GB], F32, name=f"attn_sb_{g}", tag="attn_sb")
                nc.scalar.activation(attn_sb, attn_ps, Copy)

                comb_ps = pp.tile([H, GB], F32, name=f"comb_ps_{g}", tag="gps")
                nc.tensor.matmul(comb_ps, wcT_sb[:, 0:H],
                                 encdiT_sb[:, g * GB:(g + 1) * GB], start=True, stop=False)
                nc.tensor.matmul(comb_ps, wcT_sb[:, H:2 * H],
                                 attn_sb, start=False, stop=True)
                nc.scalar.activation(xT_sb[:, g * GB:(g + 1) * GB], comb_ps, Relu,
                                     bias=bcomb_sb)

            # ---- single-step bidirectional LSTM tail ----
            for d in range(2):
                gps = []
                for ch in range(2):
                    gpp = pp1.tile([128, BS], F32, name=f"gates_{d}_{ch}", tag="lps")
                    nc.tensor.matmul(
                        gpp, wihT_sb[:, d * 256 + ch * 128:d * 256 + (ch + 1) * 128],
                        xT_sb, start=True, stop=False)
                    nc.tensor.matmul(
                        gpp, whhT_sb[:, d * 256 + ch * 128:d * 256 + (ch + 1) * 128],
                        h0T_sb[:, d * BS:(d + 1) * BS], start=False, stop=False)
                    r = d * 2 + ch
                    nc.tensor.matmul(gpp, bihr_sb[0:1, r * 128:(r + 1) * 128], ones_sb[0:1, 0:BS],
                                     start=False, stop=False)
                    nc.tensor.matmul(gpp, bhhr_sb[0:1, r * 128:(r + 1) * 128], ones_sb[0:1, 0:BS],
                                     start=False, stop=True)
                    gps.append(gpp)
                sigi = sm.tile([H, BS], F32, name=f"sigi_{d}", tag="sigi")
                nc.scalar.activation(sigi, gps[0][0:H, :], Sig)
                sigf = sm.tile([H, BS], F32, name=f"sigf_{d}", tag="sigf")
                nc.scalar.activation(sigf, gps[0][H:2 * H, :], Sig)
                tanhg = sm.tile([H, BS], F32, name=f"tanhg_{d}", tag="tanhg")
                nc.scalar.activation(tanhg, gps[1][0:H, :], Tanh)
                sigo = sm.tile([H, BS], F32, name=f"sigo_{d}", tag="sigo")
                nc.scalar.activation(sigo, gps[1][H:2 * H, :], Sig)

                t1 = sm.tile([H, BS], F32, name=f"t1_{d}", tag="t1")
                nc.vector.tensor_mul(t1, sigf, c0T_sb[:, d * BS:(d + 1) * BS])
                t2 = sm.tile([H, BS], F32, name=f"t2_{d}", tag="t2")
                nc.vector.tensor_mul(t2, sigi, tanhg)
                cT = sm.tile([H, BS], F32, name=f"cT_{d}", tag="cT")
                nc.vector.tensor_add(cT, t1, t2)
                tanhc = sm.tile([H, BS], F32, name=f"tanhc_{d}", tag="tanhc")
                nc.scalar.activation(tanhc, cT, Tanh)
                hT = sm.tile([H, BS], F32, name=f"hT_{d}", tag="hT")
                nc.vector.tensor_mul(hT, sigo, tanhc)

                # transpose back to [b, h] and write outputs
                for src, dst in ((hT, None), (cT, out2)):
                    tp = pp1.tile([BS, H], F32, name=f"tp_{d}_{0 if dst is None else 1}", tag="lps")
                    nc.tensor.transpose(tp, src, iden_sb[0:H, 0:H])
                    nat = sm.tile([BS, H], F32, name=f"nat_{d}_{0 if dst is None else 1}", tag="nat")
                    nc.scalar.activation(nat, tp, Copy)
                    if dst is None:
                        nc.sync.dma_start(out=out1[d, :, :], in_=nat)
                        nc.sync.dma_start(out=out0[:, 0, d * H:(d + 1) * H], in_=nat)
                    else:
                        nc.sync.dma_start(out=dst[d, :, :], in_=nat)

    nc.finalize()
    return nc


def kernel(h0, c0, encoder_outputs, W_attn, b_attn, W_comb, b_comb,
           w_ih_f, w_hh_f, b_ih_f, b_hh_f, w_ih_r, w_hh_r, b_ih_r, b_hh_r, di):
    global _NC, _LAST_RESULTS
    f = np.float32
    h0 = np.asarray(h0, f)
    c0 = np.asarray(c0, f)
    enc = np.ascontiguousarray(np.asarray(encoder_outputs, f))
    di_i = int(np.asarray(di))

    w1rep4 = np.ascontiguousarray(np.broadcast_to(np.tile(np.asarray(W_attn, f)[0, :EH], 4), (128, 4 * EH)))
    wcT = np.ascontiguousarray(np.asarray(W_comb, f).T.reshape(2, 128, H))
    bcomb = np.ascontiguousarray(np.asarray(b_comb, f).reshape(H, 1))
    wihT = np.ascontiguousarray(np.stack([np.asarray(w_ih_f, f).T, np.asarray(w_ih_r, f).T]))
    whhT = np.ascontiguousarray(np.stack([np.asarray(w_hh_f, f).T, np.asarray(w_hh_r, f).T]))

    def pack_bias(bf, br):
        return np.ascontiguousarray(
            np.stack([np.asarray(bf, f), np.asarray(br, f)]).reshape(4, 128))

    bihr = pack_bias(b_ih_f, b_ih_r)
    bhhr = pack_bias(b_hh_f, b_hh_r)
    h0T = np.ascontiguousarray(np.transpose(h0, (0, 2, 1)))   # (2, H, B)
    c0T = np.ascontiguousarray(np.transpose(c0, (0, 2, 1)))
    enc_diT = np.ascontiguousarray(enc[:, di_i, :].T)         # (EH, B)
    iden = np.eye(128, dtype=f)
    onesm = np.ones((128, 128), f)

    if _NC is None:
        _NC = _build()

    in_maps = []
    for i in range(M):
        sl = slice(i * BS, (i + 1) * BS)
        in_maps.append(dict(
            enc=np.ascontiguousarray(enc[sl]),
            enc_diT=np.ascontiguousarray(enc_diT[:, sl]),
            w1rep4=w1rep4, wcT=wcT, bcomb=bcomb,
            wihT=wihT, whhT=whhT, bihr=bihr, bhhr=bhhr,
            h0T=np.ascontiguousarray(h0T[:, :, sl]),
            c0T=np.ascontiguousarray(c0T[:, :, sl]),
            iden=iden, ones=onesm,
        ))

    _LAST_RESULTS = run_bass_kernel_spmd(
        _NC, in_maps, core_ids=list(range(M)), trace=TRACE)
    res = _LAST_RESULTS.results

    output = np.concatenate([res[i]["out0"] for i in range(M)], axis=0)
    h_new = np.concatenate([res[i]["out1"] for i in range(M)], axis=1)
    c_new = np.concatenate([res[i]["out2"] for i in range(M)], axis=1)
    return output, h_new, c_new
